# revision 1
# baseline (speedup 1.0000x reference)
"""Trainium2 Bass kernel for nn_MoEGate_6150393168540 (moe_routing).

Computes, for x [B=65536, D=1024], gate/expert weights [E=8, D] and biases [E]:
    gate = softmax(x @ gate_w.T + gate_b)            # [B, 8]
    keep top-k (k=2) gate values, zero the rest (no renormalization)
    expert = x @ expert_w.T + expert_b               # [B, 8]
    out = sum(gate_masked * expert, axis=1)          # [B, 1]

Strategy (8 NeuronCores, data-parallel over the batch):
  - Each core gets 8192 rows of x; weights are replicated.
  - The D-contraction needs x transposed (D on partitions). PE transposes x in
    fp32 ([128,128] blocks -> PSUM, bit-exact).
  - Scores must order-match a pure-fp32 reference (top-2 selection), so the
    matmul uses an exact fp16 Dekker split: hi = fp16(xT), lo = fp16(xT - hi)
    (ACT casts hi from PSUM, DVE computes lo). Weights are split host-side the
    same way. hi*w_hi, hi*w_lo, lo*w_hi are exact 22-bit products accumulated
    in fp32 PSUM => scores accurate to ~1e-7, at fp16 matmul speed (4x faster
    than fp32 streaming).
  - mm_hi: lhsT=[w_hi|w_lo] [128,32] -> psum rows 0:32; mm_lo: lhsT=w_hi
    [128,16] accumulates into rows 0:16 of the same group.
  - A tiny "fold" matmul (lhsT = scores [32,128-col slice], rhs = [I16;I16])
    both transposes scores back to batch-major and sums the two partial rows.
  - Postprocess per PSUM bank [128 rows, 32 groups, 16]: +bias, exp on ACT,
    top-2 threshold via a min/max tournament tree, mask, weighted sum, divide
    by the softmax denominator; DVE 32x32 block transpose to make the output
    DMA contiguous.
"""

import sys

sys.path.insert(0, "/opt/trn_rl_repo")

from contextlib import ExitStack

import numpy as np

import concourse.bass as bass
import concourse.mybir as mybir
import concourse.tile as tile

F32 = mybir.dt.float32
F16 = mybir.dt.float16
BF16 = mybir.dt.bfloat16
U16 = mybir.dt.uint16
ALU = mybir.AluOpType
AXX = mybir.AxisListType.X
EXP = mybir.ActivationFunctionType.Exp

B, D, E = 65536, 1024, 8
N_CORES = 8
B_LOC = B // N_CORES  # 8192
BLK = 512  # rows per block
DC = D // 128  # 8 contraction chunks


def split_waits(nc, max_waits: int = 1) -> int:
    """walrus here allows only one semaphore wait per instruction; hoist the
    rest into preceding single-wait NOPs on the same engine (engine streams
    execute in order, so earlier waits on the same engine are equivalent)."""
    n_split = 0
    for f in nc.m.functions:
        for bb in f.blocks:
            new = []
            for inst in bb.instructions:
                si = inst.sync_info
                if si is not None and si.on_wait and len(si.on_wait) > max_waits:
                    waits = list(si.on_wait)
                    for w in waits[:-max_waits]:
                        n_split += 1
                        nop = mybir.InstNoOp(name=f"{inst.name}-ws{n_split}")
                        nop.engine = inst.engine
                        nop.sync_info = mybir.SyncInfo(on_wait=[w], on_update=[])
                        new.append(nop)
                    inst.sync_info = mybir.SyncInfo(
                        on_wait=waits[-max_waits:], on_update=list(si.on_update or [])
                    )
                new.append(inst)
            bb.instructions = new
    return n_split


def build_module(b_loc: int = B_LOC, split: bool = True):
    assert b_loc % 4096 == 0
    nc = bass.Bass()
    x = nc.dram_tensor("x", [b_loc, D], F32, kind="ExternalInput")
    whl = nc.dram_tensor("whl", [DC, 128, 32], F16, kind="ExternalInput")
    brow = nc.dram_tensor("brow", [512], F32, kind="ExternalInput")
    fold = nc.dram_tensor("fold", [32, 16], F16, kind="ExternalInput")
    ident = nc.dram_tensor("ident", [128, 128], F32, kind="ExternalInput")
    y = nc.dram_tensor("y", [b_loc], F32, kind="ExternalOutput")

    n_blk = b_loc // BLK
    tt = nc.vector.tensor_tensor

    with tile.TileContext(nc) as tc, ExitStack() as ctx:
        consts = ctx.enter_context(tc.tile_pool(name="consts", bufs=1))
        xpool = ctx.enter_context(tc.tile_pool(name="xpool", bufs=4))
        xh_pool = ctx.enter_context(tc.tile_pool(name="xh", bufs=8))
        xl_pool = ctx.enter_context(tc.tile_pool(name="xl", bufs=8))
        z32_pool = ctx.enter_context(tc.tile_pool(name="z32", bufs=3))
        pp = ctx.enter_context(tc.tile_pool(name="pp", bufs=3))
        xt_pool = ctx.enter_context(tc.tile_pool(name="xtps", bufs=4, space="PSUM"))
        z_pool = ctx.enter_context(tc.tile_pool(name="zps", bufs=2, space="PSUM"))
        zt_pool = ctx.enter_context(tc.tile_pool(name="ztps", bufs=2, space="PSUM"))

        # const tiles allocated here; their DMAs are issued inside block 0
        # (after the identity and first x quarter) so the critical first
        # transpose isn't queued behind them on the HWDGE ring
        ident_sb = consts.tile([128, 128], F32)
        whl_sb = consts.tile([128, DC, 32], F16)
        fold_sb = consts.tile([32, 16], F16)
        bias_sb = consts.tile([128, 512], F32)

        def postprocess(zt_ps, b0):
            # zt_ps [128, 512] = [128 rows, 32 groups, 8 gate | 8 expert]
            zb = pp.tile([128, 32, 16], F32)
            nc.vector.tensor_add(
                zb, zt_ps.rearrange("p (g e) -> p g e", e=16),
                bias_sb.rearrange("p (g e) -> p g e", e=16),
            )
            g8 = zb[:, :, 0:8]
            y8 = zb[:, :, 8:16]
            p8 = pp.tile([128, 32, 8], F32)
            nc.scalar.activation(p8, g8, EXP)
            den = pp.tile([128, 32], F32)
            nc.vector.tensor_reduce(den, p8, axis=AXX, op=ALU.add)
            # top-2 threshold: tournament keeping (max, 2nd max) per segment
            h1 = pp.tile([128, 32, 4], F32)
            l1 = pp.tile([128, 32, 4], F32)
            tt(h1, p8[:, :, 0:4], p8[:, :, 4:8], op=ALU.max)
            tt(l1, p8[:, :, 0:4], p8[:, :, 4:8], op=ALU.min)
            h2 = pp.tile([128, 32, 2], F32)
            v2 = pp.tile([128, 32, 2], F32)
            u2 = pp.tile([128, 32, 2], F32)
            m2q = pp.tile([128, 32, 2], F32)
            tt(h2, h1[:, :, 0:2], h1[:, :, 2:4], op=ALU.max)
            tt(v2, h1[:, :, 0:2], h1[:, :, 2:4], op=ALU.min)
            tt(u2, l1[:, :, 0:2], l1[:, :, 2:4], op=ALU.max)
            tt(m2q, u2, v2, op=ALU.max)
            v3 = pp.tile([128, 32, 1], F32)
            u3 = pp.tile([128, 32, 1], F32)
            m2f = pp.tile([128, 32, 1], F32)
            tt(v3, h2[:, :, 0:1], h2[:, :, 1:2], op=ALU.min)
            tt(u3, m2q[:, :, 0:1], m2q[:, :, 1:2], op=ALU.max)
            tt(m2f, u3, v3, op=ALU.max)
            # mask & weighted sum
            msk = pp.tile([128, 32, 8], F32)
            tt(msk, p8, m2f.to_broadcast([128, 32, 8]), op=ALU.is_ge)
            pm = pp.tile([128, 32, 8], F32)
            tt(pm, msk, p8, op=ALU.mult)
            prod = pp.tile([128, 32, 8], F32)
            tt(prod, pm, y8, op=ALU.mult)
            num = pp.tile([128, 32], F32)
            nc.vector.tensor_reduce(num, prod, axis=AXX, op=ALU.add)
            rden = pp.tile([128, 32], F32)
            nc.vector.reciprocal(rden, den)
            outv = pp.tile([128, 32], F32)
            tt(outv, num, rden, op=ALU.mult)
            # 32x32 block transpose so each partition holds one contiguous run
            tv = pp.tile([128, 32], F32)
            nc.vector.transpose(tv, outv)
            yf = y.ap()
            for a in range(4):
                # dest[i, k] = y[b0 + 32a + 128 i + k], i,k in 0..32
                dest = bass.AP(yf.tensor, b0 + 32 * a, [[128, 32], [1, 32]])
                nc.sync.dma_start(out=dest, in_=tv[32 * a : 32 * a + 32, :])

        zt_ps = None
        state = {"zt_ps": None}

        def emit_stage(blk, c, z_ps, xh, xl):
            # hi stream (bf16 view) and lo stream (fp16), both against the
            # full [w_hi|w_lo] fp16 pair: together they accumulate the
            # complete product (hi+lo)*(w_hi+w_lo) into rows 0:32.
            nc.tensor.matmul(
                z_ps, whl_sb[:, c, :], xh.bitcast(BF16),
                start=(c == 0), stop=False,
            )
            nc.tensor.matmul(
                z_ps, whl_sb[:, c, :], xl, start=False, stop=(c == DC - 1)
            )
            if c == DC - 1:
                # block tail: scores as an exact fp16 (hi, lo) pair so the
                # fold matmuls get single-pass fp16 weight loads
                zz = z32_pool.tile([32, 1024], F16)
                nc.scalar.copy(zz[:, 0:512], z_ps)
                nc.vector.tensor_sub(zz[:, 512:1024], z_ps, zz[:, 0:512])
                bank_i = blk % 8
                if bank_i == 0:
                    state["zt_ps"] = zt_pool.tile([128, 512], F32, name="zt_ps", tag="zt_ps")
                zt_ps = state["zt_ps"]
                for j in range(4):
                    col = (bank_i * 4 + j) * 16
                    nc.tensor.matmul(
                        zt_ps[:, col : col + 16],
                        zz[:, 128 * j : 128 * j + 128],
                        fold_sb,
                        start=True,
                        stop=False,
                    )
                    nc.tensor.matmul(
                        zt_ps[:, col : col + 16],
                        zz[:, 512 + 128 * j : 512 + 128 * j + 128],
                        fold_sb,
                        start=False,
                        stop=True,
                    )
                if bank_i == 7:
                    postprocess(zt_ps, (blk // 8) * 4096)

        pending = []
        for blk in range(n_blk):
            r0 = blk * BLK
            x_blk = xpool.tile([128, 4, D], F32, name="x_blk", tag="x_blk")
            xin = x.ap()[r0 : r0 + BLK, :].rearrange("(j p) d -> p j d", p=128)
            if blk == 0:
                # identity first (tiny), then d-quarters of x so chunk c=0
                # can start after one quarter; remaining consts afterwards
                nc.sync.dma_start(out=ident_sb, in_=ident.ap())
                for q in range(4):
                    nc.sync.dma_start(
                        out=x_blk[:, :, 256 * q : 256 * q + 256],
                        in_=xin[:, :, 256 * q : 256 * q + 256],
                    )
                nc.sync.dma_start(out=whl_sb, in_=whl.ap().transpose([1, 0, 2]))
                nc.sync.dma_start(out=fold_sb, in_=fold.ap())
                nc.gpsimd.dma_start(
                    out=bias_sb,
                    in_=brow.ap().unsqueeze(0).to_broadcast([128, 512]),
                )
            else:
                nc.sync.dma_start(out=x_blk, in_=xin)
            z_ps = z_pool.tile([32, 512], F32)
            for c in range(DC):
                xt_ps = xt_pool.tile([128, 512], F32)
                for j in range(4):
                    nc.tensor.transpose(
                        xt_ps[:, 128 * j : 128 * j + 128],
                        x_blk[:, j, 128 * c : 128 * c + 128],
                        ident_sb,
                    )
                # hi = bf16 truncation of xT: a pure u16 bit-slice copy on ACT
                # (psum -> sbuf); lo = fp16(xT - hi) on DVE (one psum input).
                xt_hi_view = (
                    xt_ps.bitcast(U16)
                    .rearrange("p (k two) -> p k two", two=2)[:, :, 1]
                )
                xh = xh_pool.tile([128, 512], U16)
                nc.scalar.copy(xh, xt_hi_view)
                xl = xl_pool.tile([128, 512], F16)
                nc.vector.tensor_sub(xl, xt_ps, xh.bitcast(BF16))
                pending.append((blk, c, z_ps, xh, xl))
                if len(pending) > 6:
                    emit_stage(*pending.pop(0))
        for args in pending:
            emit_stage(*args)

    if split:
        split_waits(nc)
    return nc


def host_inputs(gate_w, gate_b, expert_w, expert_b):
    """Host-side prep of the small replicated tensors."""
    W = np.concatenate([gate_w, expert_w], axis=0).astype(np.float32)  # [16, D]
    WT = W.T  # [D, 16]
    w_hi = WT.astype(np.float16)
    w_lo = (WT - w_hi.astype(np.float32)).astype(np.float16)
    whl = np.empty((DC, 128, 32), dtype=np.float16)
    for c in range(DC):
        whl[c, :, 0:16] = w_hi[128 * c : 128 * (c + 1), :]
        whl[c, :, 16:32] = w_lo[128 * c : 128 * (c + 1), :]
    bcat = np.concatenate([gate_b, expert_b]).astype(np.float32)  # [16]
    brow = np.tile(bcat, 32)  # [512]
    fold = np.concatenate([np.eye(16), np.eye(16)], axis=0).astype(np.float16)
    ident = np.eye(128, dtype=np.float32)
    return {"whl": whl, "brow": brow, "fold": fold, "ident": ident}


_NC_CACHE = {}


def kernel(x, gate_w, gate_b, expert_w, expert_b, k):
    assert int(k) == 2
    x = np.ascontiguousarray(np.asarray(x, dtype=np.float32))
    assert x.shape == (B, D)

    from concourse.bass_utils import run_bass_kernel_spmd

    if B_LOC not in _NC_CACHE:
        _NC_CACHE[B_LOC] = build_module(B_LOC)
    nc = _NC_CACHE[B_LOC]

    common = host_inputs(
        np.asarray(gate_w, np.float32),
        np.asarray(gate_b, np.float32),
        np.asarray(expert_w, np.float32),
        np.asarray(expert_b, np.float32),
    )
    in_maps = [
        {**common, "x": x[i * B_LOC : (i + 1) * B_LOC]} for i in range(N_CORES)
    ]
    import os

    trace = bool(os.environ.get("MOE_TRACE"))
    if trace:
        _ensure_ntff_hook()
    res = run_bass_kernel_spmd(
        nc, in_maps, core_ids=list(range(N_CORES)), trace=trace
    )
    global LAST_RESULT
    LAST_RESULT = res
    out = np.concatenate([r["y"] for r in res.results])
    return out.reshape(B, 1).astype(np.float32)


LAST_RESULT = None


def _ensure_ntff_hook():
    """Register the axon NTFF profile hook if the antenv shim is missing
    (lets run_bass_kernel_spmd(trace=True) capture HW timing)."""
    try:
        import antenv.axon_hooks  # noqa: F401

        return
    except ImportError:
        pass
    try:
        import types

        import antenv
        from trn_agent_boot.trn_boot import _ntff_profile_via_ctypes

        mod = types.ModuleType("antenv.axon_hooks")
        _h = [None]
        mod.set_axon_ntff_profile_hook = lambda h: _h.__setitem__(0, h)
        mod.get_axon_ntff_profile_hook = lambda: _h[0]
        sys.modules["antenv.axon_hooks"] = mod
        antenv.axon_hooks = mod
        mod.set_axon_ntff_profile_hook(
            _ntff_profile_via_ctypes("/opt/axon/libaxon_pjrt.so")
        )
    except Exception as e:  # profiling is best-effort
        print(f"ntff hook setup failed: {e}")


if __name__ == "__main__":
    rng = np.random.default_rng(0)
    s = 1.0 / np.sqrt(D)
    inputs = {
        "x": rng.standard_normal((B, D), dtype=np.float32),
        "gate_w": rng.uniform(-s, s, (E, D)).astype(np.float32),
        "gate_b": rng.uniform(-s, s, E).astype(np.float32),
        "expert_w": rng.uniform(-s, s, (E, D)).astype(np.float32),
        "expert_b": rng.uniform(-s, s, E).astype(np.float32),
        "k": 2,
    }
    got = kernel(**inputs)
    print("kernel output:", got.shape, got.dtype, got[:4, 0])

